# revision 31
# baseline (speedup 1.0000x reference)
"""BiRWKV layer kernel for 8 Trainium2 NeuronCores.

Strategy (data-parallel over B=8, one batch element per core), v5:
  - (channel, time) layout on chip: channels on the 128 SBUF partitions
    (C=512 -> 4 blocks), time on the free dim.
  - All projections are bf16 matmuls (fp8e4 DoubleRow was tried and is
    2x faster on the PE, but costs ~3.4e-2 relative error vs the 2e-2
    budget, so it was reverted).
  - WKV runs UNSTABILIZED (mathematically equal to the reference's
    log-sum-exp form; values stay in range since |w|*T <= ~28, k~N(0,1)):
        den_t = d*den_{t-1} + e^{k_t};  num_t = d*num_{t-1} + e^{k_t} v_t
        y_t   = (num_{t-1} + e^{k_t+u} v_t) / (den_{t-1} + e^{k_t+u})
  - The den/num recurrences run on the DVE via single 1024-wide
    tensor_tensor_scan per (pair, cb) -- 2.5 ns/col vs 3.4 at width
    512; the backward direction uses reversed access patterns.
  - sigmoid is folded into the denominator: y = sig(r)*num/den
    = num / (den * (1 + e^{-r})), so the only activation functions used
    are exp/ln/copy which share ONE ACT table set (no 1.28us reloads).
  - Engine balance: ACT does exp/ln/copy (psum eviction) in
    stream-major order (k-exp first) so the in-order ACT queue feeds
    the scan backbone promptly; DVE does scans + the v-multiply
    (straight from PSUM) + the two fused STT combines; Pool does the
    SBUF-only adds/muls; the output eviction stays on ACT.
"""

import numpy as np
import ml_dtypes

B, T, C = 8, 4096, 512
TT = 512           # time tile (psum width)
CB = 4             # channel blocks
PW = 2 * TT        # pair width for SBUF-side elementwise
NP = T // PW       # 4 pairs

_CACHE = {}


def _apply_tile_patches():
    """walrus in this container rejects instructions with >1 sync wait
    ("Too many sync wait commands"). Split excess waits onto same-engine
    nop carriers, and do the same for the TileContext tail drain."""
    import concourse.tile as tile_mod
    from concourse import mybir
    from concourse.vector_clock import ScopedClock

    if getattr(tile_mod, "_wait_split_patched", False):
        return
    MAXW = 1

    _orig_add = tile_mod.TileContext._add_instruction

    def _split_add(self, inst):
        si = inst.sync_info
        if si is not None and si.on_wait and len(si.on_wait) > MAXW:
            waits = list(si.on_wait)
            k = 0
            while len(waits) > MAXW:
                chunk, waits = waits[:MAXW], waits[MAXW:]
                carrier = mybir.InstNoOp(
                    name=f"{inst.name}_wsplit{k}",
                    engine=inst.engine,
                    bass_nofuse=True,
                    sync_info=mybir.SyncInfo(on_wait=chunk, on_update=[]),
                )
                k += 1
                _orig_add(self, carrier)
            inst.sync_info = mybir.SyncInfo(
                on_wait=waits, on_update=list(si.on_update)
            )
        return _orig_add(self, inst)

    def _drain_and_barrier(self, tick_clock, wait_clock):
        drain_inst = self.nc.sync.drain()
        wait_clock.add_sem_waits(
            drain_inst.ins, ScopedClock({None: tick_clock.global_clock})
        )
        si = drain_inst.ins.sync_info
        if si is not None and si.on_wait and len(si.on_wait) > MAXW:
            waits = list(si.on_wait)
            drain_inst.ins.sync_info = mybir.SyncInfo(
                on_wait=waits[:MAXW], on_update=list(si.on_update)
            )
            rest = waits[MAXW:]
            while rest:
                chunk, rest = rest[:MAXW], rest[MAXW:]
                n = self.nc.sync.nop(nofuse=True)
                n.ins.sync_info = mybir.SyncInfo(on_wait=chunk, on_update=[])

        self.nc.all_engine_barrier()
        assert self.sems is not None
        popped = self.nc._tile_sem_poison_stack.pop()
        assert popped is self._sem_poison
        self.nc.clear_and_free_semaphores(list(self.sems.allocated().values()))
        self.nc.all_engine_barrier()

    tile_mod.TileContext._add_instruction = _split_add
    tile_mod.TileContext._drain_and_barrier = _drain_and_barrier
    tile_mod._wait_split_patched = True


def _build_nc():
    import concourse.bass as bass
    import concourse.tile as tile
    from concourse import mybir

    _apply_tile_patches()

    f32 = mybir.dt.float32
    bf16 = mybir.dt.bfloat16
    Alu = mybir.AluOpType
    Act = mybir.ActivationFunctionType

    nc = bass.Bass()

    xT = nc.dram_tensor("xT", [C, T], bf16, kind="ExternalInput")
    wnames = ["w_rf", "w_kf", "w_vf", "w_rb", "w_kb", "w_vb"]
    wdram = {
        n: nc.dram_tensor(n, [128, 4 * C], bf16, kind="ExternalInput")
        for n in wnames
    }
    wout_d = nc.dram_tensor("wout", [128, 8 * C], bf16, kind="ExternalInput")
    consts_d = nc.dram_tensor("consts", [C, 6], f32, kind="ExternalInput")
    out_d = nc.dram_tensor("y", [T, C], f32, kind="ExternalOutput")
    ypf_s = nc.dram_tensor("ypf_s", [C, T], bf16)  # fwd y_pre staging (HBM)

    with tile.TileContext(nc) as tc:
        with (
            tc.tile_pool(name="wp", bufs=1) as wp,
            tc.tile_pool(name="cst", bufs=1) as cst,
            tc.tile_pool(name="chain", bufs=2) as chainp,
            tc.tile_pool(name="xt", bufs=2) as xtp,
            tc.tile_pool(name="wk", bufs=1) as wkp,
            tc.tile_pool(name="ps", bufs=1, space="PSUM") as psp,
        ):
            # ---- resident weights & constants ----
            # issue order matters: fwd-phase weights + consts first so the
            # first pair's compute can start; wout/bwd weights land later.
            wt = {}
            for n in ["w_kf", "w_vf", "w_rf"]:
                wt[n] = wp.tile([128, 4 * C], bf16, tag=n, name=n)
                nc.sync.dma_start(wt[n][:], wdram[n][:])
            u_t, eu_t, dec_t = {}, {}, {}
            for cb in range(CB):
                sl = slice(cb * 128, (cb + 1) * 128)
                ct = cst.tile([128, 6], f32, tag=f"cst{cb}", name=f"cst{cb}")
                nc.sync.dma_start(ct[:], consts_d[sl, :])
                for j, d in enumerate(("f", "b")):
                    u_t[(d, cb)] = ct[:, 3 * j + 0: 3 * j + 1]
                    eu_t[(d, cb)] = ct[:, 3 * j + 1: 3 * j + 2]
                    dec_t[(d, cb)] = ct[:, 3 * j + 2: 3 * j + 3]
            for n in ["w_kb", "w_vb", "w_rb"]:
                wt[n] = wp.tile([128, 4 * C], bf16, tag=n, name=n)
            wout = wp.tile([128, 8 * C], bf16, name="wout")

            def run_phase(d):
                fwd = d == "f"
                if not fwd:
                    # bwd weights land while the fwd phase computes, after
                    # the fwd pairs' x tiles are already in the DMA queue.
                    for n in ["w_kb", "w_vb", "w_rb"]:
                        nc.sync.dma_start(wt[n][:], wdram[n][:])
                    nc.sync.dma_start(wout[:], wout_d[:])
                wr, wk, wv = wt["w_r" + d], wt["w_k" + d], wt["w_v" + d]
                pairs = list(range(NP)) if fwd else list(reversed(range(NP)))
                chains = {}

                def chain_buf(cb, kind):
                    # seed slot written on Pool (for the den_prev/num_prev
                    # shifted reads); the scan's init reads the previous
                    # tile's tail directly so no copy sits in the DVE stream.
                    key = (cb, kind)
                    t = chainp.tile([128, PW + 1], bf16,
                                    tag=f"ch_{kind}{cb}",
                                    name=f"ch_{kind}{cb}")
                    prev = chains.get(key)
                    chains[key] = t
                    if fwd:
                        if prev is None:
                            nc.gpsimd.memset(t[:, 0:1], 0.0)
                            init = 0.0
                        else:
                            nc.gpsimd.tensor_copy(t[:, 0:1],
                                                  prev[:, PW: PW + 1])
                            init = prev[:, PW: PW + 1]
                    else:
                        if prev is None:
                            nc.gpsimd.memset(t[:, PW: PW + 1], 0.0)
                            init = 0.0
                        else:
                            nc.gpsimd.tensor_copy(t[:, PW: PW + 1],
                                                  prev[:, 0:1])
                            init = prev[:, 0:1]
                    return t, init

                def emit_inv(bag):
                    invbs = {}
                    for cb in range(CB):
                        invbs[cb] = wkp.tile([128, PW], bf16, tag="invb",
                                             bufs=4, name="invb")
                        nc.scalar.activation(invbs[cb][:], bag["lnbs"][cb][:],
                                             Act.Exp, scale=-1.0)
                    bag["invbs"] = invbs

                def emit_yc(bag):
                    p0 = bag["p0"]
                    nmrs, invbs = bag["nmrs"], bag["invbs"]
                    ypb_tiles = {}
                    for cb in range(CB):
                        yb = wkp.tile([128, PW], bf16, tag="ypb",
                                      bufs=6, name="ypb")
                        nc.gpsimd.tensor_mul(yb[:], nmrs[cb][:],
                                             invbs[cb][:])
                        if fwd:
                            nc.sync.dma_start(
                                ypf_s[cb * 128:(cb + 1) * 128, p0: p0 + PW],
                                yb[:])
                        else:
                            ypb_tiles[cb] = yb
                    if not fwd:
                        ypfl = {}
                        for cb in range(CB):
                            ypfl[cb] = wkp.tile([128, PW], bf16,
                                                tag=f"ypfl{cb}", bufs=2,
                                                name=f"ypfl{cb}")
                            nc.sync.dma_start(
                                ypfl[cb][:],
                                ypf_s[cb * 128:(cb + 1) * 128, p0: p0 + PW])
                        for m in range(PW // 128):
                            t0 = p0 + m * 128
                            pso = psp.tile([128, C], f32, tag="po",
                                           bufs=2, name="pso")
                            for cb in range(CB):
                                nc.tensor.matmul(
                                    pso[:],
                                    ypfl[cb][:, m * 128: (m + 1) * 128],
                                    wout[:, cb * C: (cb + 1) * C],
                                    start=(cb == 0), stop=False)
                            for cb in range(CB):
                                nc.tensor.matmul(
                                    pso[:],
                                    ypb_tiles[cb][:, m * 128: (m + 1) * 128],
                                    wout[:, (4 + cb) * C: (5 + cb) * C],
                                    start=False, stop=(cb == 3))
                            osb = wkp.tile([128, C], f32, tag="osb",
                                           bufs=2, name="osb")
                            nc.scalar.copy(osb[:], pso[:])
                            nc.sync.dma_start(
                                out_d[t0: t0 + 128, :], osb[:])

                for pr in pairs:
                    p0 = pr * PW
                    # ---------------- part A ----------------
                    # bf16 x tiles (r proj rhs) + fp8 x tiles (k/v DR rhs)
                    xts = {}
                    for half, tt in enumerate((2 * pr, 2 * pr + 1)):
                        t0 = tt * TT
                        xt = xtp.tile([128, 4, TT], bf16, tag=f"xth{half}",
                                      bufs=2, name=f"xth{half}")
                        nc.sync.dma_start(
                            xt[:],
                            xT[:, t0: t0 + TT].rearrange(
                                "(k p) t -> p k t", k=4))
                        xts[half] = xt
                    stash = {}
                    for cb in range(CB):
                        pss = {}
                        for cls, nb in (("k", 2), ("r", 2), ("v", 2)):
                            for half in range(2):
                                pss[(cls, half)] = psp.tile(
                                    [128, TT], f32, tag=f"p{cls}", bufs=nb,
                                    name=f"ps{cls}")
                        for cls, w in (("k", wk), ("r", wr), ("v", wv)):
                            for kb in range(4):
                                wsl = w[:, kb * C + cb * 128:
                                        kb * C + cb * 128 + 128]
                                for half in range(2):
                                    nc.tensor.matmul(
                                        pss[(cls, half)][:], wsl,
                                        xts[half][:, kb, :],
                                        start=(kb == 0), stop=(kb == 3))
                        er2 = wkp.tile([128, PW], bf16, tag="er2",
                                       bufs=4, name="er2")
                        ekt = wkp.tile([128, PW], bf16, tag="ek",
                                       bufs=3, name="ek")
                        ekvt = wkp.tile([128, PW], bf16, tag="ekv",
                                        bufs=3, name="ekv")
                        ekbt = wkp.tile([128, PW], bf16, tag="ekb",
                                        bufs=3, name="ekb")
                        # stream-major ACT order matching matmul completion
                        # (k first, r last) so scan-critical ekt never queues
                        # behind er/vs ops on the in-order ACT engine.
                        for half in range(2):
                            nc.scalar.activation(
                                ekt[:, half * TT:(half + 1) * TT],
                                pss[("k", half)][:], Act.Exp)
                        for half in range(2):
                            nc.scalar.activation(
                                ekbt[:, half * TT:(half + 1) * TT],
                                pss[("k", half)][:], Act.Exp,
                                bias=u_t[(d, cb)])
                        for half in range(2):
                            nc.scalar.activation(
                                er2[:, half * TT:(half + 1) * TT],
                                pss[("r", half)][:], Act.Exp,
                                bias=0.0, scale=-1.0)
                        # full-width (1024) scans, one per (pair, cb, kind);
                        # DVE stream stays scan/TT-only here so the scan
                        # backbone never head-of-line blocks on Pool/ACT.
                        decbc = dec_t[(d, cb)].broadcast_to([128, PW])
                        denb, den_init = chain_buf(cb, "den")
                        numb, num_init = chain_buf(cb, "num")
                        if fwd:
                            nc.vector.tensor_tensor_scan(
                                denb[:, 1: 1 + PW], decbc,
                                ekt[:], den_init, Alu.mult, Alu.add)
                        else:
                            nc.vector.tensor_tensor_scan(
                                denb[:, 0: PW][:, ::-1], decbc,
                                ekt[:][:, ::-1], den_init,
                                Alu.mult, Alu.add)
                        for half in range(2):
                            hs = slice(half * TT, (half + 1) * TT)
                            nc.vector.tensor_mul(ekvt[:, hs], ekt[:, hs],
                                                 pss[("v", half)][:])
                        if fwd:
                            nc.vector.tensor_tensor_scan(
                                numb[:, 1: 1 + PW], decbc,
                                ekvt[:], num_init, Alu.mult, Alu.add)
                        else:
                            nc.vector.tensor_tensor_scan(
                                numb[:, 0: PW][:, ::-1], decbc,
                                ekvt[:][:, ::-1], num_init,
                                Alu.mult, Alu.add)
                        if fwd:
                            den_prev = denb[:, 0:PW]
                            num_prev = numb[:, 0:PW]
                        else:
                            den_prev = denb[:, 1: 1 + PW]
                            num_prev = numb[:, 1: 1 + PW]
                        stash[cb] = (ekbt, ekvt, er2, den_prev, num_prev)

                    # ---- B1: dnm (Pool) + nmr (DVE STT) ----
                    dnms, nmrs = {}, {}
                    for cb in range(CB):
                        ekbt, ekvt, er2, den_prev, num_prev = stash[cb]
                        dnms[cb] = wkp.tile([128, PW], bf16, tag="dnm",
                                            bufs=4, name="dnm")
                        nc.gpsimd.tensor_add(dnms[cb][:], ekbt[:],
                                             den_prev[:])
                        nmrs[cb] = wkp.tile([128, PW], bf16, tag="nmr",
                                            bufs=8, name="nmr")
                        nc.vector.scalar_tensor_tensor(
                            nmrs[cb][:], ekvt[:], eu_t[(d, cb)],
                            num_prev[:], Alu.mult, Alu.add)
                    # ---- B3/C for the PREVIOUS pair (deferred one pair so
                    # Pool's in-order stream runs dnm(p) before y(p-1)) ----
                    # ---- B2: dnm2 (DVE STT) + ln (ACT) ----
                    lnbs = {}
                    for cb in range(CB):
                        _, _, er2, _, _ = stash[cb]
                        dnm2 = wkp.tile([128, PW], bf16, tag="dnm2",
                                        bufs=4, name="dnm2")
                        nc.vector.scalar_tensor_tensor(
                            dnm2[:], er2[:], 1.0, dnms[cb][:],
                            Alu.add, Alu.mult)
                        lnbs[cb] = wkp.tile([128, PW], f32, tag="lnb",
                                            bufs=5, name="lnb")
                        nc.scalar.activation(lnbs[cb][:], dnm2[:], Act.Ln)
                    bag = dict(p0=p0, nmrs=nmrs, lnbs=lnbs)
                    emit_inv(bag)
                    emit_yc(bag)

            run_phase("f")
            run_phase("b")

    return nc


def _host_prep(x, W_rkv, W_out, time_decay, time_first, time_decay_rev,
               time_first_rev):
    bf16 = ml_dtypes.bfloat16
    f32 = np.float32

    Wr = W_rkv.reshape(C, 2, 3, C)
    pieces = {
        "w_rf": Wr[:, 0, 0], "w_kf": Wr[:, 0, 1], "w_vf": Wr[:, 0, 2],
        "w_rb": Wr[:, 1, 0], "w_kb": Wr[:, 1, 1], "w_vb": Wr[:, 1, 2],
    }
    wmaps = {}
    for n, p in pieces.items():
        wmaps[n] = np.ascontiguousarray(
            p.reshape(4, 128, C).transpose(1, 0, 2).reshape(128, 4 * C)
        ).astype(bf16)

    Wo = W_out.reshape(8, 128, C).transpose(1, 0, 2).reshape(128, 8 * C)
    wout = np.ascontiguousarray(Wo).astype(bf16)

    u_f = time_first.astype(f32)
    u_b = time_first_rev.astype(f32)
    eu_f = np.exp(time_first.astype(np.float64)).astype(f32)
    eu_b = np.exp(time_first_rev.astype(np.float64)).astype(f32)
    dec_f = np.exp(-np.exp(time_decay.astype(np.float64))).astype(f32)
    dec_b = np.exp(-np.exp(time_decay_rev.astype(np.float64))).astype(f32)
    consts = np.ascontiguousarray(
        np.stack([u_f, eu_f, dec_f, u_b, eu_b, dec_b], axis=1)
    ).astype(f32)

    shared = dict(wout=wout, consts=consts, **wmaps)
    in_maps = []
    for b in range(B):
        m = dict(shared)
        m["xT"] = np.ascontiguousarray(x[b].T).astype(bf16)
        in_maps.append(m)
    return in_maps


def kernel(x, W_rkv, W_out, time_decay, time_first, time_decay_rev,
           time_first_rev, _trace=False):
    from concourse.bass_utils import run_bass_kernel_spmd

    x = np.asarray(x, dtype=np.float32)
    W_rkv = np.asarray(W_rkv, dtype=np.float32)
    W_out = np.asarray(W_out, dtype=np.float32)
    time_decay = np.asarray(time_decay, dtype=np.float32)
    time_first = np.asarray(time_first, dtype=np.float32)
    time_decay_rev = np.asarray(time_decay_rev, dtype=np.float32)
    time_first_rev = np.asarray(time_first_rev, dtype=np.float32)

    if "nc" not in _CACHE:
        _CACHE["nc"] = _build_nc()
    nc = _CACHE["nc"]

    in_maps = _host_prep(x, W_rkv, W_out, time_decay, time_first,
                         time_decay_rev, time_first_rev)
    res = run_bass_kernel_spmd(
        nc, in_maps, core_ids=list(range(B)), trace=_trace
    )
    _CACHE["last_result"] = res
    out = np.stack([res.results[b]["y"].astype(np.float32) for b in range(B)])
    return out
